# revision 7
# baseline (speedup 1.0000x reference)
"""Trainium2 Bass kernel for a BERT-style self-attention block (B=2, S=4096,
H=768, NH=12) sharded over 8 NeuronCores.

Sharding: data-parallel over batch (2) x query-block parallel (4) = 8 cores.
Each core computes K/V for the full sequence of its batch and a disjoint
1024-query slice of the output; no collectives, LN epilogue fully local.

Key design points vs the bf16 baseline:
  - All PE operands are fp8 e4m3 (host pre-scales weights and x by 32 to
    clear the e4m3 subnormal zone; the 32*32*8 score scale is folded into
    the softmax exp argument, and the o-proj output scale 1024 is folded
    into the residual, which LayerNorm is invariant to).
  - 256-contraction matmuls (q/k/v/o projections, P@V context) run in
    MatmulPerfMode.DoubleRow (2 fp8 MACs/cell/cycle).
  - The softmax exp (50M elements/core -- the real bottleneck) is split
    between ScalarE (ACT Exp -> fp8) and VectorE (Schraudolph bit-trick:
    one tensor_scalar mult+add emitting int8 bit patterns that ARE the
    e4m3 encoding of exp), ratio tuned by dve16/16.
  - Softmax denominator from a ones-column appended to V (row 64 of the
    context psum); reciprocal broadcast across partitions via DRAM bounce.
"""

import math

import numpy as np
import ml_dtypes

B, S, H, NH = 2, 4096, 768, 12
HD = H // NH  # 64
NCORES = 8
SQ = S // 4  # 1024 queries per core
LN_EPS = 1e-12
CW = 32.0  # host scale on weights and x into fp8-normal range
ALPHA = 1.0 / (8.0 * CW * CW)  # exp(ALPHA * s_raw) = softmax argument
SCH_A = ALPHA * 8.0 / math.log(2.0)  # Schraudolph slope (e4m3: 8 steps/octave)
SCH_B = 56.0  # e4m3 exponent bias 7 * 8
VPAD = 80  # V cols: 64 dims + 1 ones + 15 pad (DoubleRow weight step % 16)

_BUILD_CACHE = {}


def build(stage="full", general=False, dve16=6, psk=2, psc=2, ptb=6, xtb=3,
          rdb_=3, keng="s", qeng="s", veng="v"):
    key = (stage, general, dve16, psk, psc, ptb, xtb, rdb_, keng, qeng, veng)
    if key in _BUILD_CACHE:
        return _BUILD_CACHE[key]

    import concourse.mybir as mybir
    import concourse.tile as tile
    from concourse import bacc

    dt = mybir.dt
    f32, f8 = dt.float32, dt.float8e4
    AF = mybir.ActivationFunctionType
    OP = mybir.AluOpType
    PM = mybir.MatmulPerfMode

    EC = H // 128  # 6 contraction chunks over H
    OC = H // 128  # 6 output-channel chunks
    NSC512 = S // 512  # 512-wide s chunks (K/V projection)
    NC256 = S // 256  # 256-key chunks (attention/DoubleRow ctx)
    NST = S // 128  # 128-wide key tiles (mask layout)
    QT = 512
    NQT = SQ // QT
    NQCH = SQ // 512
    NP2 = NH // 2
    NSTQ = SQ // 128

    nc = bacc.Bacc("TRN2", target_bir_lowering=False, debug=False)

    xT = nc.dram_tensor("xT", [H, S], f8, kind="ExternalInput")
    xTq = nc.dram_tensor("xTq", [H, SQ], f8, kind="ExternalInput")
    wqT = nc.dram_tensor("wqT", [H, H], f8, kind="ExternalInput")
    wkT = nc.dram_tensor("wkT", [H, H], f8, kind="ExternalInput")
    wvT = nc.dram_tensor("wvT", [H, H], f8, kind="ExternalInput")
    woT = nc.dram_tensor("woT", [H, H], f8, kind="ExternalInput")
    qb = nc.dram_tensor("qb", [H], f32, kind="ExternalInput")
    kb = nc.dram_tensor("kb", [H], f32, kind="ExternalInput")
    vb = nc.dram_tensor("vb", [H], f32, kind="ExternalInput")
    mask = nc.dram_tensor("mask", [S], f32, kind="ExternalInput")
    maskA = nc.dram_tensor("maskA", [S], f32, kind="ExternalInput")
    xres = nc.dram_tensor("xres", [SQ, H], f32, kind="ExternalInput")
    lng = nc.dram_tensor("lng", [H], f32, kind="ExternalInput")
    lnb = nc.dram_tensor("lnb", [H], f32, kind="ExternalInput")
    out = nc.dram_tensor("out", [SQ, H], f32, kind="ExternalOutput")

    def drmm(pout, lhsT, rhs, first, last):
        # DoubleRow accumulation over EC pairs: lhsT/rhs sliced [128, 2, *]
        nc.tensor.matmul(pout, lhsT, rhs, start=first, stop=last,
                         perf_mode=PM.DoubleRow)

    def store(engine, dst, src, bias_ap):
        """psum fp32 -> SBUF fp8 (+ optional bias) on the chosen engine."""
        if general and bias_ap is not None:
            nc.vector.tensor_scalar_add(dst, src, bias_ap)
        elif engine == "s":
            nc.scalar.activation(dst, src, AF.Copy)
        else:
            nc.vector.tensor_copy(dst, src)

    def emit_qkv(nc, pools):
        (xtp, psK, wq_sb, wk_sb, wv_sb, kt_sb, qt_sb, v_sb) = pools
        xTq_r = xTq.rearrange("(c p) s -> p c s", p=128)
        for qc in range(NQCH):
            xtq = xtp.tile([128, EC, 512], f8, tag="xt")
            nc.sync.dma_start(xtq[:], xTq_r[:, :, qc * 512 : (qc + 1) * 512])
            for oc in range(OC):
                pq = psK.tile([128, 512], f32, tag="pk")
                for e in range(EC // 2):
                    drmm(pq[:], wq_sb[:, 2 * e : 2 * e + 2, oc * 128 : (oc + 1) * 128],
                         xtq[:, 2 * e : 2 * e + 2, :], e == 0, e == EC // 2 - 1)
                store(qeng, qt_sb[:, oc, qc * 512 : (qc + 1) * 512], pq[:],
                      qb_sb[:, oc : oc + 1] if general else None)
        xT_r = xT.rearrange("(c p) s -> p c s", p=128)
        for sc in range(NSC512):
            xt = xtp.tile([128, EC, 512], f8, tag="xt")
            nc.sync.dma_start(xt[:], xT_r[:, :, sc * 512 : (sc + 1) * 512])
            for oc in range(OC):
                pk = psK.tile([128, 512], f32, tag="pk")
                for e in range(EC // 2):
                    drmm(pk[:], wk_sb[:, 2 * e : 2 * e + 2, oc * 128 : (oc + 1) * 128],
                         xt[:, 2 * e : 2 * e + 2, :], e == 0, e == EC // 2 - 1)
                store(keng, kt_sb[:, oc, sc * 512 : (sc + 1) * 512], pk[:],
                      kb_sb[:, oc : oc + 1] if general else None)
            for t4 in range(4):
                st = sc * 4 + t4
                c, ko = st // 2, st % 2
                pva = psK.tile([128, 512], f32, tag="pk")
                pvb = psK.tile([128, 512], f32, tag="pk")
                for e in range(EC // 2):
                    xs = xt[:, 2 * e : 2 * e + 2, t4 * 128 : (t4 + 1) * 128]
                    drmm(pva[:], xs, wv_sb[:, 2 * e : 2 * e + 2, 0:512],
                         e == 0, e == EC // 2 - 1)
                    drmm(pvb[:, 0 : H - 512], xs,
                         wv_sb[:, 2 * e : 2 * e + 2, 512:H],
                         e == 0, e == EC // 2 - 1)
                if general:
                    nc.vector.tensor_tensor(
                        v_sb[:, c, ko, 0:8, 0:HD],
                        pva.rearrange("p (h d) -> p h d", d=HD),
                        vb_bc[:, 0:512].rearrange("p (h d) -> p h d", d=HD),
                        OP.add)
                    nc.vector.tensor_tensor(
                        v_sb[:, c, ko, 8:NH, 0:HD],
                        pvb[:, 0 : H - 512].rearrange("p (h d) -> p h d", d=HD),
                        vb_bc[:, 512:H].rearrange("p (h d) -> p h d", d=HD),
                        OP.add)
                else:
                    store(veng, v_sb[:, c, ko, 0:8, 0:HD],
                          pva.rearrange("p (h d) -> p h d", d=HD), None)
                    store(veng, v_sb[:, c, ko, 8:NH, 0:HD],
                          pvb[:, 0 : H - 512].rearrange("p (h d) -> p h d", d=HD),
                          None)

    def emit_attention(nc, pools):
        (ptp, rdp, rddr, psA, psC, kt_sb, qt_sb, v_sb) = pools
        do_ctx = stage not in ("attn1",)
        do_norm = stage not in ("attn1", "attn2")
        u = 0
        for qt in range(NQT):
            q0 = qt * QT
            for p2 in range(NP2):
                cx = [psC.tile([VPAD, QT], f32, tag="ctx", name=f"ctx{i}")
                      for i in range(2)]
                for c in range(NC256):
                    sp = [psA.tile([128, 2, QT], f32, tag="big", name=f"sp{i}")
                          for i in range(2)]
                    for kh in range(2):
                        k0 = c * 256 + kh * 128
                        nc.tensor.matmul(
                            sp[0][:, kh, :],
                            kt_sb[0:64, p2, k0 : k0 + 128],
                            qt_sb[0:64, p2, q0 : q0 + QT],
                            start=True, stop=True)
                        nc.tensor.matmul(
                            sp[1][:, kh, :],
                            kt_sb[64:128, p2, k0 : k0 + 128],
                            qt_sb[64:128, p2, q0 : q0 + QT],
                            start=True, stop=True)
                    pts = []
                    for i01 in range(2):
                        pt = ptp.tile([128, 2, QT], f8, tag="pt")
                        pts.append(pt)
                        use_dve = ((u * dve16) % 16) < dve16
                        u += 1
                        if general:
                            for kh in range(2):
                                mcol = slice(2 * c + kh, 2 * c + kh + 1)
                                if use_dve:
                                    nc.vector.tensor_scalar(
                                        pt[:, kh, :].bitcast(mybir.dt.int8),
                                        sp[i01][:, kh, :],
                                        SCH_A, maskA_sb[:, mcol],
                                        OP.mult, OP.add)
                                else:
                                    nc.scalar.activation(
                                        pt[:, kh, :], sp[i01][:, kh, :], AF.Exp,
                                        bias=mask_sb[:, mcol], scale=ALPHA)
                        elif use_dve:
                            nc.vector.tensor_scalar(
                                pt[:].bitcast(mybir.dt.int8), sp[i01][:],
                                SCH_A, SCH_B, OP.mult, OP.add)
                        else:
                            nc.scalar.activation(pt[:], sp[i01][:], AF.Exp,
                                                 scale=ALPHA)
                    if stage == "attn1":
                        nc.gpsimd.dma_start(
                            out[0:128, 0:256].bitcast(mybir.dt.uint8),
                            pts[0].rearrange("p a q -> p (a q)").bitcast(mybir.dt.uint8))
                    if do_ctx:
                        for i01 in range(2):
                            nc.tensor.matmul(
                                cx[i01][:], v_sb[:, c, :, 2 * p2 + i01, :],
                                pts[i01][:], start=(c == 0), stop=(c == NC256 - 1),
                                perf_mode=mybir.MatmulPerfMode.DoubleRow)
                if stage == "attn2":
                    dbg = rdp.tile([VPAD, QT], f32, tag="dbg")
                    nc.vector.tensor_copy(dbg[:], cx[0][:])
                    nc.gpsimd.dma_start(out[0:VPAD, 0:QT], dbg[:])
                for i01 in range(2) if do_norm else []:
                    h = 2 * p2 + i01
                    rd = rdp.tile([1, QT], f32, tag="rd")
                    nc.vector.reciprocal_approx_fast(rd[:], cx[i01][HD : HD + 1, :])
                    rdd = rddr.tile([1, QT], f32, tag="rdd")
                    nc.sync.dma_start(rdd[:], rd[:])
                    rdb = rdp.tile([64, QT], f32, tag="rdb")
                    nc.sync.dma_start(rdb[:], rdd[:].to_broadcast((64, QT)))
                    nc.vector.tensor_tensor(
                        ctxT[(h % 2) * 64 : (h % 2) * 64 + 64, h // 2,
                             q0 : q0 + QT],
                        cx[i01][0:HD, :], rdb[:], OP.mult)

    def emit_tail(nc, tc):
        with tc.tile_pool(name="tailc", bufs=1) as tpc, \
             tc.tile_pool(name="tailw", bufs=3) as tpw, \
             tc.tile_pool(name="ys", bufs=NSTQ) as yp, \
             tc.tile_pool(name="ps3", bufs=2, space="PSUM") as ps3:
            wo_sb = tpc.tile([128, EC, H], f8, tag="wo")
            eps_ap = tpc.tile([128, 1], f32, tag="eps")
            ss_all = tpc.tile([128, NSTQ], f32, tag="ss")
            negmu_all = tpc.tile([128, NSTQ], f32, tag="negmu")
            std_all = tpc.tile([128, NSTQ], f32, tag="std")
            rstd_all = tpc.tile([128, NSTQ], f32, tag="rstd")
            nc.sync.dma_start(wo_sb[:], woT.rearrange("(c p) o -> p c o", p=128))
            if general:
                lng_bc = tpc.tile([128, H], f32, tag="lngbc")
                lnb_bc = tpc.tile([128, H], f32, tag="lnbbc")
                nc.sync.dma_start(lng_bc[:], lng[None, :].to_broadcast((128, H)))
                nc.sync.dma_start(lnb_bc[:], lnb[None, :].to_broadcast((128, H)))
            nc.vector.memset(eps_ap[:], float(LN_EPS))
            ys = []
            for st in range(NSTQ):
                po = ps3.tile([128, H], f32, tag="po")
                for e in range(EC // 2):
                    lh = ctxT[:, 2 * e : 2 * e + 2, st * 128 : (st + 1) * 128]
                    drmm(po[:, 0:512], lh, wo_sb[:, 2 * e : 2 * e + 2, 0:512],
                         e == 0, e == EC // 2 - 1)
                    drmm(po[:, 512:H], lh, wo_sb[:, 2 * e : 2 * e + 2, 512:H],
                         e == 0, e == EC // 2 - 1)
                xr = tpw.tile([128, H], f32, tag="xr")
                nc.sync.dma_start(xr[:], xres[st * 128 : (st + 1) * 128, :])
                y = yp.tile([128, H], f32, tag="y")
                ysum = tpw.tile([128, 1], f32, tag="ysum")
                nc.vector.tensor_tensor(y[:], po[:], xr[:], OP.add)
                nc.vector.reduce_sum(ysum[:], y[:], axis=mybir.AxisListType.X)
                nc.vector.tensor_scalar_mul(
                    negmu_all[:, st : st + 1], ysum[:], -1.0 / H)
                sq = tpw.tile([128, H], f32, tag="scratch")
                nc.scalar.activation(
                    sq[:], y[:], AF.Square,
                    bias=negmu_all[:, st : st + 1], scale=1.0,
                    accum_out=ss_all[:, st : st + 1])
                ys.append(y)
            nc.scalar.activation(std_all[:], ss_all[:], AF.Sqrt,
                                 bias=eps_ap[:, 0:1], scale=1.0 / H)
            nc.vector.reciprocal(rstd_all[:], std_all[:])
            for st in range(NSTQ):
                t1 = tpw.tile([128, H], f32, tag="scratch")
                nc.vector.tensor_scalar(
                    t1[:], ys[st][:],
                    negmu_all[:, st : st + 1], rstd_all[:, st : st + 1],
                    OP.add, OP.mult)
                if general:
                    t2 = tpw.tile([128, H], f32, tag="scratch")
                    nc.vector.tensor_tensor(t2[:], t1[:], lng_bc[:], OP.mult)
                    ot = tpw.tile([128, H], f32, tag="scratch")
                    nc.vector.tensor_tensor(ot[:], t2[:], lnb_bc[:], OP.add)
                else:
                    ot = t1
                nc.sync.dma_start(out[st * 128 : (st + 1) * 128, :], ot[:])

    with tile.TileContext(nc) as tc:
        with tc.tile_pool(name="persist", bufs=1) as pp:
            ctxT = pp.tile([128, EC, SQ], f8, tag="ctxT")
            mask_sb = pp.tile([128, NST], f32, tag="mask")
            nc.sync.dma_start(mask_sb[:], mask.rearrange("(c p) -> p c", p=128))
            if general:
                qb_sb = pp.tile([128, OC], f32, tag="qb")
                kb_sb = pp.tile([128, OC], f32, tag="kb")
                vb_bc = pp.tile([128, H], f32, tag="vbbc")
                maskA_sb = pp.tile([128, NST], f32, tag="maskA")
                nc.sync.dma_start(qb_sb[:], qb.rearrange("(c p) -> p c", p=128))
                nc.sync.dma_start(kb_sb[:], kb.rearrange("(c p) -> p c", p=128))
                nc.sync.dma_start(vb_bc[:], vb[None, :].to_broadcast((128, H)))
                nc.sync.dma_start(maskA_sb[:],
                                  maskA.rearrange("(c p) -> p c", p=128))

            with tc.tile_pool(name="bulk", bufs=1) as bulk:
                kt_sb = bulk.tile([128, OC, S], f8, tag="kt")
                qt_sb = bulk.tile([128, OC, SQ], f8, tag="qt")
                v_sb = bulk.tile([128, NC256, 2, NH, VPAD], f8, tag="v")
                wq_sb = bulk.tile([128, EC, H], f8, tag="wq")
                wk_sb = bulk.tile([128, EC, H], f8, tag="wk")
                wv_sb = bulk.tile([128, EC, H], f8, tag="wv")
                nc.sync.dma_start(wq_sb[:], wqT.rearrange("(c p) o -> p c o", p=128))
                nc.sync.dma_start(wk_sb[:], wkT.rearrange("(c p) o -> p c o", p=128))
                nc.sync.dma_start(wv_sb[:], wvT.rearrange("(c p) o -> p c o", p=128))
                nc.vector.memset(v_sb[:, :, :, :, HD : HD + 1], 1.0)
                nc.vector.memset(v_sb[:, :, :, :, HD + 1 : VPAD], 0.0)
                with tc.tile_pool(name="xtp", bufs=xtb) as xtp, \
                     tc.tile_pool(name="ptp", bufs=ptb) as ptp, \
                     tc.tile_pool(name="rdp", bufs=rdb_) as rdp, \
                     tc.tile_pool(name="rddr", bufs=3, space="DRAM") as rddr, \
                     tc.tile_pool(name="psA", bufs=2, space="PSUM") as psA, \
                     tc.tile_pool(name="psK", bufs=psk, space="PSUM") as psK, \
                     tc.tile_pool(name="psC", bufs=psc, space="PSUM") as psC:
                    emit_qkv(nc, (xtp, psK, wq_sb, wk_sb, wv_sb,
                                  kt_sb, qt_sb, v_sb))
                    if stage == "proj":
                        nc.gpsimd.dma_start(
                            out[0:128, 0:192].bitcast(mybir.dt.uint8),
                            qt_sb[:, :, 0:128].bitcast(mybir.dt.uint8))
                        nc.gpsimd.dma_start(
                            out[128:256, 0:192].bitcast(mybir.dt.uint8),
                            kt_sb[:, :, 0:128].bitcast(mybir.dt.uint8))
                    else:
                        emit_attention(nc, (ptp, rdp, rddr, psA, psC,
                                            kt_sb, qt_sb, v_sb))
            if stage == "attn":
                nc.gpsimd.dma_start(out[0:128, 0:192].bitcast(mybir.dt.uint8),
                                    ctxT[:, :, 0:128].bitcast(mybir.dt.uint8))
            if stage == "full":
                emit_tail(nc, tc)

    nc.compile()
    _BUILD_CACHE[key] = nc
    return nc


def make_in_maps(inputs):
    """Host-side sharding: slice/transpose/scale/cast the full inputs into
    the 8 per-core input maps."""
    f8 = ml_dtypes.float8_e4m3
    hs = np.ascontiguousarray(np.asarray(inputs["hidden_states"], np.float32))
    am = np.asarray(inputs["attention_mask"], np.float32)
    q_w = np.asarray(inputs["q_w"], np.float32)
    k_w = np.asarray(inputs["k_w"], np.float32)
    v_w = np.asarray(inputs["v_w"], np.float32)
    o_w = np.asarray(inputs["o_w"], np.float32)
    q_b = np.asarray(inputs["q_b"], np.float32)
    k_b = np.asarray(inputs["k_b"], np.float32)
    v_b = np.asarray(inputs["v_b"], np.float32)
    o_b = np.asarray(inputs["o_b"], np.float32)
    ln_g = np.asarray(inputs["ln_g"], np.float32)
    ln_b = np.asarray(inputs["ln_b"], np.float32)

    wqT_a = np.ascontiguousarray((q_w.T * CW).astype(f8))
    wkT_a = np.ascontiguousarray((k_w.T * CW).astype(f8))
    wvT_a = np.ascontiguousarray((v_w.T * CW).astype(f8))
    woT_a = np.ascontiguousarray((o_w.T * CW).astype(f8))
    qbs = (q_b * CW).astype(np.float32)
    kbs = (k_b * CW).astype(np.float32)
    vbs = (v_b * CW).astype(np.float32)

    nb = hs.shape[0]
    xT_full = [np.ascontiguousarray(hs[b].T.astype(f8)) for b in range(nb)]
    groups = NCORES // nb

    in_maps = []
    for c in range(NCORES):
        b, j = c // groups, c % groups
        sl = slice(j * SQ, (j + 1) * SQ)
        m = np.ascontiguousarray(am[b, 0, 0])
        in_maps.append({
            "xT": xT_full[b],
            "xTq": np.ascontiguousarray(xT_full[b][:, sl]),
            "wqT": wqT_a, "wkT": wkT_a, "wvT": wvT_a, "woT": woT_a,
            "qb": qbs, "kb": kbs, "vb": vbs,
            "mask": m,
            "maskA": (m * (8.0 / math.log(2.0)) + SCH_B).astype(np.float32),
            "xres": np.ascontiguousarray(
                (hs[b, sl] + o_b[None, :]) * (CW * CW)),
            "lng": ln_g, "lnb": ln_b,
        })
    return in_maps


def _needs_general(inputs):
    return bool(
        np.any(np.asarray(inputs["attention_mask"]))
        or np.any(np.asarray(inputs["q_b"]))
        or np.any(np.asarray(inputs["k_b"]))
        or np.any(np.asarray(inputs["v_b"]))
        or np.any(np.asarray(inputs["ln_g"]) != 1.0)
        or np.any(np.asarray(inputs["ln_b"]))
    )


def run_cores(inputs, trace=False, **kwargs):
    from concourse.bass_utils import run_bass_kernel_spmd

    build_kwargs = {
        k: kwargs.pop(k)
        for k in ("stage", "general", "dve16", "psk", "psc", "ptb", "xtb",
                  "rdb_", "keng", "qeng", "veng")
        if k in kwargs
    }
    build_kwargs.setdefault("general", _needs_general(inputs))
    nc = build(**build_kwargs)
    in_maps = make_in_maps(inputs)
    res = run_bass_kernel_spmd(
        nc, in_maps, core_ids=list(range(NCORES)), trace=trace, **kwargs)
    nb = np.asarray(inputs["hidden_states"]).shape[0]
    groups = NCORES // nb
    out = np.empty((nb, S, H), np.float32)
    for c in range(NCORES):
        b, j = c // groups, c % groups
        out[b, j * SQ : (j + 1) * SQ] = res.results[c]["out"]
    return out, res


def kernel(**inputs):
    out, _ = run_cores(inputs, trace=False)
    return out
